# revision 1
# baseline (speedup 1.0000x reference)
"""Trainium2 Bass kernel for nn_LstmCellS (matrix-state LSTM cell).

Math (per gate g in [f, i, o, c]):
    pre[g] = hidden_u @ Ww[g]^T - x @ Wd[g]^T + hidden_s @ Wu[g]^T + (Bw+Bd+Bu)[g]
    f, i, o = sigmoid(pre[0..2]);  gg = tanh(pre[3])
    c     = f * hidden_c + i * gg
    out_s = o * tanh(c)

Sharding: tensor-parallel over the output axis p (flattened (a, b), S^2 = 4096
-> 512 per core).  Every core sees the full batch and full contraction but only
its 512-wide output slice of every gate, so the whole LSTM epilogue is local —
no collectives.  Host concatenates the 8 output slices.

Per-core matmul: out[n_tile(128), p(512)] accumulated over 40 contraction tiles
of 128, where the contraction axis is the concatenation [hidden_s (4096),
hidden_u (512), x (512)] = 5120 and the weight rows are [Wu, Ww, -Wd].
Stationary operand = transposed activations (shared by all 4 gates), moving
operand = transposed weights.  Operands are cast to bf16 on the host (fp32
accumulation in PSUM); 8 PSUM banks hold the 4 gates x 2 batch tiles; the bias
is folded in via a K=1 matmul of ones^T @ bias that also warms up the PE while
the first weight slab is still in flight.

Weights stream as 1 MiB slabs ([128, 4096] bf16 = 2 contraction tiles x 4
gates) with 6 slots — this measured ~410 GB/s sustained vs ~240 GB/s for
512 KiB/3-slot streaming.
"""

import sys

for _p in ("/root/.axon_site/_ro/trn_rl_repo", "/opt/trn_rl_repo"):
    if _p not in sys.path:
        sys.path.append(_p)

import ml_dtypes
import numpy as np

B = 256          # batch
S2 = 4096        # S*S (flattened matrix state)
U = 512          # hidden_u size
I = 512          # input size
QC = S2 + U + I  # contraction length (5120)
QT = QC // 128   # contraction tiles (40)
NT = B // 128    # batch tiles (2)
NCORES = 8
PSH = S2 // NCORES  # output slice per core (512)
ACH = 4          # activation chunks (A loaded in 4 DMAs)
CHQ = QT // ACH  # q-tiles per activation chunk (10)
NSLAB = QT // 2  # weight slabs (2 q-tiles each, 20)
WBUFS = 8        # weight slab slots in SBUF

MM_DT = "bfloat16"  # matmul operand dtype: bfloat16 | float32 | float32r

_cache: dict = {}


def _mm_np(mm_dt):
    return ml_dtypes.bfloat16 if mm_dt == "bfloat16" else np.float32


def _build(mm_dt):
    """Build and compile the per-core Bass module (same NEFF on all cores)."""
    import concourse.tile as tile
    import concourse.mybir as mybir
    from concourse import bacc

    f32 = mybir.dt.float32
    mdt = getattr(mybir.dt, mm_dt)
    AF = mybir.ActivationFunctionType

    nc = bacc.Bacc("TRN2", target_bir_lowering=False, debug=False,
                   enable_asserts=False, num_devices=NCORES)

    A_d = nc.dram_tensor("A", [128, QT * B], mdt, kind="ExternalInput")
    W_d = nc.dram_tensor("W", [NSLAB, 128, 2 * 4 * PSH], mdt, kind="ExternalInput")
    B_d = nc.dram_tensor("BIAS", [1, 4 * PSH], mdt, kind="ExternalInput")
    H_d = nc.dram_tensor("HC", [128, NT * PSH], f32, kind="ExternalInput")
    OS_d = nc.dram_tensor("OS", [NT, 128, PSH], f32, kind="ExternalOutput")
    CO_d = nc.dram_tensor("CO", [NT, 128, PSH], f32, kind="ExternalOutput")

    with tile.TileContext(nc) as tc:
        with (
            tc.tile_pool(name="apool", bufs=1) as apool,
            tc.tile_pool(name="wpool", bufs=WBUFS) as wpool,
            tc.tile_pool(name="cpool", bufs=1) as cpool,
            tc.tile_pool(name="epool", bufs=2) as epool,
            tc.tile_pool(name="pspool", bufs=1, space="PSUM") as pspool,
        ):
            # PSUM accumulators: one bank per (gate, batch-tile)
            psum = [
                pspool.tile([128, PSH], f32, tag=f"ps{g}_{n}", name=f"ps{g}_{n}")
                for g in range(4) for n in range(NT)
            ]

            # HAM warmup: the PE clock-gate only opens after ~3.4us of
            # sustained matmul activity, and the first weight slab doesn't
            # land until ~9us.  Burn the dead startup window on dummy matmuls
            # into psum[0] (later reset by its start=True bias matmul) so the
            # real stream runs at 2.4 GHz from the first slab.
            scr = cpool.tile([128, 128 + PSH], mdt, tag="scr")
            nc.gpsimd.memset(scr[:], 0.0)
            for _ in range(5):
                nc.tensor.matmul(
                    psum[0][:], scr[:, :128], scr[:, 128:],
                    start=True, stop=True, skip_group_check=True)

            # Bias via K=1 matmul: psum[g,n] <- ones[1,128]^T @ bias[1,512].
            # Runs as soon as the tiny bias DMA lands; warms up the PE while
            # the first weight slab is still streaming in.
            bias_t = cpool.tile([1, 4 * PSH], mdt, tag="bias")
            nc.scalar.dma_start(bias_t[:], B_d.ap()[:])
            ones_t = cpool.tile([1, 128], mdt, tag="ones")
            nc.gpsimd.memset(ones_t[:], 1.0)
            for n in range(NT):
                for g in range(4):
                    nc.tensor.matmul(
                        psum[g * NT + n][:], ones_t[:],
                        bias_t[:, g * PSH:(g + 1) * PSH],
                        start=True, stop=False)

            # Activation chunks interleaved with their weight slabs so the
            # first matmuls start as early as possible.  The first chunk and
            # first slab are split into smaller pieces so the very first
            # matmuls only wait for ~0.4 MB of DMA instead of ~1.7 MB.
            a0a = apool.tile([128, 2 * B], mdt, tag="a0a", name="a0a")
            nc.scalar.dma_start(a0a[:], A_d.ap()[:, 0:2 * B])
            w0 = []
            for h in range(2):
                wh = wpool.tile([128, 4 * PSH], mdt, tag=f"w0{h}", name=f"w0{h}")
                nc.sync.dma_start(
                    wh[:], W_d.ap()[0][:, h * 4 * PSH:(h + 1) * 4 * PSH])
                w0.append(wh)
            a0b = apool.tile([128, (CHQ - 2) * B], mdt, tag="a", bufs=3, name="a0b")
            nc.scalar.dma_start(a0b[:], A_d.ap()[:, 2 * B:CHQ * B])
            a_tiles = [None] * ACH

            def lhs_for(qt, n):
                if qt < 2:
                    return a0a[:, qt * B + n * 128: qt * B + (n + 1) * 128]
                ch = qt // CHQ
                if ch == 0:
                    off = qt - 2
                    return a0b[:, off * B + n * 128: off * B + (n + 1) * 128]
                off = qt - ch * CHQ
                return a_tiles[ch][:, off * B + n * 128: off * B + (n + 1) * 128]

            def emit_mms(qt, wtile, base):
                for n in range(NT):
                    lhs = lhs_for(qt, n)
                    for g in range(4):
                        nc.tensor.matmul(
                            psum[g * NT + n][:],
                            lhs,
                            wtile[:, base + g * PSH: base + (g + 1) * PSH],
                            start=False,
                            stop=(qt == QT - 1),
                        )

            emit_mms(0, w0[0], 0)
            emit_mms(1, w0[1], 0)
            for ch in range(ACH):
                if ch > 0:
                    at = apool.tile([128, CHQ * B], mdt, tag="a", bufs=3, name=f"a{ch}")
                    nc.scalar.dma_start(
                        at[:], A_d.ap()[:, ch * CHQ * B:(ch + 1) * CHQ * B])
                    a_tiles[ch] = at
                for j in range(max(1, ch * NSLAB // ACH), (ch + 1) * NSLAB // ACH):
                    wt = wpool.tile([128, 2 * 4 * PSH], mdt, tag="w", name=f"w{j}")
                    nc.sync.dma_start(wt[:], W_d.ap()[j])
                    for h in range(2):
                        emit_mms(2 * j + h, wt, h * 4 * PSH)
                if ch == 1:
                    # hidden_c is only needed in the epilogue; stream it after
                    # the first half of the weights.
                    hc_t = cpool.tile([128, NT * PSH], f32, tag="hc")
                    nc.scalar.dma_start(hc_t[:], H_d.ap()[:])

            # Epilogue: gates straight out of PSUM, then the cell update.
            for n in range(NT):
                acts = []
                for g in range(4):
                    act = epool.tile([128, PSH], f32, tag=f"act{g}", name=f"act{g}_{n}")
                    nc.scalar.activation(
                        act[:], psum[g * NT + n][:],
                        AF.Tanh if g == 3 else AF.Sigmoid)
                    acts.append(act)
                f_a, i_a, o_a, g_a = acts
                fhc = epool.tile([128, PSH], f32, tag="fhc", name=f"fhc{n}")
                nc.vector.tensor_mul(fhc[:], f_a[:], hc_t[:, n * PSH:(n + 1) * PSH])
                ig = epool.tile([128, PSH], f32, tag="ig", name=f"ig{n}")
                nc.vector.tensor_mul(ig[:], i_a[:], g_a[:])
                c_t = epool.tile([128, PSH], f32, tag="ct", name=f"ct{n}")
                nc.vector.tensor_add(c_t[:], fhc[:], ig[:])
                nc.sync.dma_start(CO_d.ap()[n], c_t[:])
                th = epool.tile([128, PSH], f32, tag="th", name=f"th{n}")
                nc.scalar.activation(th[:], c_t[:], AF.Tanh)
                os_t = epool.tile([128, PSH], f32, tag="ost", name=f"ost{n}")
                nc.vector.tensor_mul(os_t[:], o_a[:], th[:])
                nc.sync.dma_start(OS_d.ap()[n], os_t[:])

    nc.compile()
    return nc


def _get_nc(mm_dt):
    if mm_dt not in _cache:
        _cache[mm_dt] = _build(mm_dt)
    return _cache[mm_dt]


def _prep_in_maps(x, hidden_s, hidden_u, hidden_c, Wd, Wu, Ww, Bd, Bu, Bw, mm_dt):
    mnp = _mm_np(mm_dt)

    # Activations, transposed: A_T[k, n], k = [hs (4096) | hu (512) | x (512)]
    A = np.concatenate(
        [hidden_s.reshape(B, S2), hidden_u, x], axis=1).astype(mnp)      # [B, QC]
    A_sb = np.ascontiguousarray(
        A.T.reshape(QT, 128, B).transpose(1, 0, 2)).reshape(128, QT * B)

    # Weights, transposed to [k, p] with gates interleaved in the free dim,
    # packed two contraction tiles per 1 MiB slab.
    WuT = Wu.reshape(4, S2, S2).astype(mnp).transpose(0, 2, 1)           # [4,S2,S2]
    WwT = Ww.reshape(4, S2, U).astype(mnp).transpose(0, 2, 1)            # [4,U,S2]
    WdT = (-Wd.reshape(4, S2, I)).astype(mnp).transpose(0, 2, 1)         # [4,I,S2]
    WT = np.concatenate([WuT, WwT, WdT], axis=1)                         # [4,QC,S2]
    W_r = WT.reshape(4, NSLAB, 2, 128, S2).transpose(1, 3, 2, 0, 4)      # [NS,128,2,4,S2]

    bias = (Bw + Bd + Bu).reshape(4, S2).astype(np.float32)
    hc = hidden_c.reshape(NT, 128, S2).astype(np.float32)

    in_maps = []
    for c in range(NCORES):
        p0 = c * PSH
        W_c = np.ascontiguousarray(
            W_r[..., p0:p0 + PSH]).reshape(NSLAB, 128, 2 * 4 * PSH)
        b_c = np.ascontiguousarray(bias[:, p0:p0 + PSH]).reshape(1, 4 * PSH).astype(mnp)
        h_c = np.ascontiguousarray(
            hc[..., p0:p0 + PSH].transpose(1, 0, 2)).reshape(128, NT * PSH)
        in_maps.append({"A": A_sb, "W": W_c, "BIAS": b_c, "HC": h_c})
    return in_maps


def _run(inputs, mm_dt=None, trace=False, trace_kwargs=None):
    from concourse.bass_utils import run_bass_kernel_spmd

    mm_dt = mm_dt or MM_DT
    nc = _get_nc(mm_dt)
    in_maps = _prep_in_maps(mm_dt=mm_dt, **inputs)
    res = run_bass_kernel_spmd(
        nc, in_maps, core_ids=list(range(NCORES)),
        trace=trace, **(trace_kwargs or {}))

    out_s = np.empty((B, S2), np.float32)
    c_out = np.empty((B, S2), np.float32)
    for c in range(NCORES):
        p0 = c * PSH
        out_s[:, p0:p0 + PSH] = res.results[c]["OS"].reshape(B, PSH)
        c_out[:, p0:p0 + PSH] = res.results[c]["CO"].reshape(B, PSH)
    return (out_s.reshape(B, 64, 64), c_out.reshape(B, 64, 64)), res


def kernel(**inputs):
    inputs = {k: np.asarray(v) for k, v in inputs.items()}
    (out_s, c_out), _ = _run(inputs)
    return (out_s, c_out)



# revision 2
# speedup vs baseline: 1.3430x; 1.3430x over previous
"""Trainium2 Bass kernel for nn_LstmCellS (matrix-state LSTM cell).

Math (per gate g in [f, i, o, c]):
    pre[g] = hidden_u @ Ww[g]^T - x @ Wd[g]^T + hidden_s @ Wu[g]^T + (Bw+Bd+Bu)[g]
    f, i, o = sigmoid(pre[0..2]);  gg = tanh(pre[3])
    c     = f * hidden_c + i * gg
    out_s = o * tanh(c)

Sharding: tensor-parallel over the output axis p (flattened (a, b), S^2 = 4096
-> 512 per core).  Every core sees the full batch and full contraction but only
its 512-wide output slice of every gate, so the whole LSTM epilogue is local —
no collectives.  Host concatenates the 8 output slices.

Mixed precision: the f/i/o gates go through sigmoid (derivative <= 0.25), so
fp8-e4m3 quantization noise in their pre-activations is strongly attenuated;
the c-candidate gate goes through tanh (derivative ~1) and dominates the error.
Hence f/i/o matmuls run in fp8 DoubleRow mode (2 contraction rows per PE cell
per cycle) and the candidate gate runs in bf16.  Host-simulated rel err 1.34e-2
vs the 2e-2 budget.  fp8 operands are pre-scaled on host (activations x16,
weights x4096 — the weights are uniform in +-1/64, right at e4m3's subnormal
boundary) and the 2^-16 is folded into the epilogue activation's scale.

Gate-major stream order f, i, candidate, o: c = f*hc + i*g is computed on the
scalar/vector engines while the o-gate matmuls still run, leaving only
sigmoid(o) * tanh(c) + the output DMA in the tail.

All operands are SBUF-resident (~150 KiB of the 208 KiB per partition): weights
13.1 MiB + activations 3.9 MiB stream in ordered by first use, with no tile
recycling hazards.
"""

import sys

for _p in ("/root/.axon_site/_ro/trn_rl_repo", "/opt/trn_rl_repo"):
    if _p not in sys.path:
        sys.path.append(_p)

import ml_dtypes
import numpy as np

B = 256          # batch
S2 = 4096        # S*S (flattened matrix state)
U = 512          # hidden_u size
I = 512          # input size
QC = S2 + U + I  # contraction length (5120)
QT = QC // 128   # contraction tiles (40)
KP = QT // 2     # fp8 DoubleRow k-pairs (20)
NT = B // 128    # batch tiles (2)
NCORES = 8
PSH = S2 // NCORES  # output slice per core (512)

SA = 16.0        # fp8 activation scale
SW = 4096.0      # fp8 weight scale
SINV = 1.0 / (SA * SW)

F8 = ml_dtypes.float8_e4m3  # TRN float8e4: bias 7, max normal +-240
BF = ml_dtypes.bfloat16

_cache: dict = {}


def _build():
    """Build and compile the per-core Bass module (same NEFF on all cores)."""
    import concourse.tile as tile
    import concourse.mybir as mybir
    from concourse import bacc

    f32 = mybir.dt.float32
    bf16 = mybir.dt.bfloat16
    fp8 = mybir.dt.float8e4
    AF = mybir.ActivationFunctionType
    DR = mybir.MatmulPerfMode.DoubleRow

    nc = bacc.Bacc("TRN2", target_bir_lowering=False, debug=False,
                   enable_asserts=False, num_devices=NCORES)

    A8_d = nc.dram_tensor("A8", [128, QT * B], fp8, kind="ExternalInput")
    AB_d = nc.dram_tensor("AB", [128, QT * B], bf16, kind="ExternalInput")
    W8_d = nc.dram_tensor("W8", [3, 128, KP * 2 * PSH], fp8, kind="ExternalInput")
    WB_d = nc.dram_tensor("WB", [128, QT * PSH], bf16, kind="ExternalInput")
    B_d = nc.dram_tensor("BIAS", [1, 4 * PSH], bf16, kind="ExternalInput")
    H_d = nc.dram_tensor("HC", [128, NT * PSH], bf16, kind="ExternalInput")
    OS_d = nc.dram_tensor("OS", [NT, 128, PSH], f32, kind="ExternalOutput")
    CO_d = nc.dram_tensor("CO", [NT, 128, PSH], f32, kind="ExternalOutput")

    with tile.TileContext(nc) as tc:
        with (
            tc.tile_pool(name="apool", bufs=1) as apool,
            tc.tile_pool(name="wpool", bufs=1) as wpool,
            tc.tile_pool(name="cpool", bufs=1) as cpool,
            tc.tile_pool(name="epool", bufs=2) as epool,
            tc.tile_pool(name="pspool", bufs=1, space="PSUM") as pspool,
        ):
            # PSUM accumulators: bank per (gate, batch-tile); gate ids f0 i1 o2 g3
            psum = [
                pspool.tile([128, PSH], f32, tag=f"ps{g}_{n}", name=f"ps{g}_{n}")
                for g in range(4) for n in range(NT)
            ]

            # HAM warmup: the PE clock-gate only opens after ~3.4us of
            # sustained matmul activity.  Burn the DMA-wait startup window on
            # dummy matmuls into psum[0] (later reset by its start=True bias
            # matmul) so the real stream runs at 2.4 GHz from the first slab.
            # memsets on the vector engine — gpsimd dispatch was measured ~6us
            # late on this kernel.
            scr = cpool.tile([128, 128 + PSH], bf16, tag="scr")
            nc.vector.memset(scr[:], 0.0)
            for _ in range(5):
                nc.tensor.matmul(
                    psum[0][:], scr[:, :128], scr[:, 128:],
                    start=True, stop=True, skip_group_check=True)

            # Bias via K=1 matmul: psum[g,n] <- ones[1,128]^T @ bias[1,512].
            # Gates f/i/o carry the 2^16 fp8 pre-scale in the host-side bias.
            bias_t = cpool.tile([1, 4 * PSH], bf16, tag="bias")
            nc.scalar.dma_start(bias_t[:], B_d.ap()[:])
            ones_t = cpool.tile([1, 128], bf16, tag="ones")
            nc.vector.memset(ones_t[:], 1.0)
            for n in range(NT):
                for g in range(4):
                    nc.tensor.matmul(
                        psum[g * NT + n][:], ones_t[:],
                        bias_t[:, g * PSH:(g + 1) * PSH],
                        start=True, stop=False)

            # SBUF-resident operands.  DMAs are issued in first-use order:
            # activations + small tensors on the scalar queue, weights on sync.
            a8 = apool.tile([128, QT, B], fp8, tag="a8", name="a8")
            ab = apool.tile([128, QT * B], bf16, tag="ab", name="ab")
            w8 = [
                wpool.tile([128, KP, 2, PSH], fp8, tag=f"w8_{g}", name=f"w8_{g}")
                for g in range(3)
            ]
            wbg = wpool.tile([128, QT * PSH], bf16, tag="wbg", name="wbg")
            hc_t = cpool.tile([128, NT * PSH], bf16, tag="hc")

            def dma_w8(g, j0, j1):
                nc.sync.dma_start(
                    w8[g][:, j0:j1, :, :],
                    W8_d.ap()[g][:, j0 * 2 * PSH:j1 * 2 * PSH])

            def dma_a8(q0, q1):
                nc.scalar.dma_start(a8[:, q0:q1, :], A8_d.ap()[:, q0 * B:q1 * B])

            def dma_ab(q0, q1):
                nc.scalar.dma_start(
                    ab[:, q0 * B:q1 * B], AB_d.ap()[:, q0 * B:q1 * B])

            def dma_wbg(q0, q1):
                nc.sync.dma_start(
                    wbg[:, q0 * PSH:q1 * PSH], WB_d.ap()[:, q0 * PSH:q1 * PSH])

            # f-gate needs a8 + w8[0] across the whole contraction within the
            # first ~10us of the stream; front-load those.
            dma_a8(0, 4)
            dma_w8(0, 0, 2)
            dma_w8(0, 2, 6)
            dma_a8(4, 16)
            dma_w8(0, 6, 12)
            dma_a8(16, 28)
            dma_w8(0, 12, 20)
            dma_a8(28, 40)
            for (j0, j1) in ((0, 7), (7, 14), (14, 20)):
                dma_w8(1, j0, j1)
            for (q0, q1) in ((0, 14), (14, 27), (27, 40)):
                dma_ab(q0, q1)
            for (q0, q1) in ((0, 10), (10, 20), (20, 30), (30, 40)):
                dma_wbg(q0, q1)
            nc.scalar.dma_start(hc_t[:], H_d.ap()[:])
            for (j0, j1) in ((0, 10), (10, 20)):
                dma_w8(2, j0, j1)

            # fp8 DoubleRow stream for a sigmoid gate g (psum bank id == g).
            def fp8_gate(g):
                for j in range(KP):
                    for n in range(NT):
                        nc.tensor.matmul(
                            psum[g * NT + n][:],
                            a8[:, 2 * j:2 * j + 2, n * 128:(n + 1) * 128],
                            w8[g][:, j, :, :],
                            start=False, stop=(j == KP - 1), perf_mode=DR)

            fp8_gate(0)                           # f
            fp8_gate(1)                           # i
            for q in range(QT):                   # candidate (bf16, gate id 3)
                for n in range(NT):
                    nc.tensor.matmul(
                        psum[3 * NT + n][:],
                        ab[:, q * B + n * 128:q * B + (n + 1) * 128],
                        wbg[:, q * PSH:(q + 1) * PSH],
                        start=False, stop=(q == QT - 1))
            fp8_gate(2)                           # o (last: c computes under it)

            # Epilogue.  Scalar-engine program order matches data-ready order:
            # f, i (early), candidate tanh + tanh(c) (during o), sigmoid(o).
            f_a, i_a, g_a, c_t, th = [], [], [], [], []
            for n in range(NT):
                t = epool.tile([128, PSH], bf16, tag="fa", name=f"fa{n}")
                nc.scalar.activation(t[:], psum[0 * NT + n][:], AF.Sigmoid,
                                     scale=SINV)
                f_a.append(t)
            for n in range(NT):
                t = epool.tile([128, PSH], bf16, tag="ia", name=f"ia{n}")
                nc.scalar.activation(t[:], psum[1 * NT + n][:], AF.Sigmoid,
                                     scale=SINV)
                i_a.append(t)
            for n in range(NT):
                t = epool.tile([128, PSH], bf16, tag="ga", name=f"ga{n}")
                nc.scalar.activation(t[:], psum[3 * NT + n][:], AF.Tanh)
                g_a.append(t)
            for n in range(NT):
                fhc = epool.tile([128, PSH], f32, tag="fhc", name=f"fhc{n}")
                nc.vector.tensor_mul(
                    fhc[:], f_a[n][:], hc_t[:, n * PSH:(n + 1) * PSH])
                ig = epool.tile([128, PSH], f32, tag="ig", name=f"ig{n}")
                nc.vector.tensor_mul(ig[:], i_a[n][:], g_a[n][:])
                ct = epool.tile([128, PSH], f32, tag="ct", name=f"ct{n}")
                nc.vector.tensor_add(ct[:], fhc[:], ig[:])
                c_t.append(ct)
                nc.sync.dma_start(CO_d.ap()[n], ct[:])
            for n in range(NT):
                t = epool.tile([128, PSH], bf16, tag="th", name=f"th{n}")
                nc.scalar.activation(t[:], c_t[n][:], AF.Tanh)
                th.append(t)
            for n in range(NT):
                o_a = epool.tile([128, PSH], bf16, tag="oa", name=f"oa{n}")
                nc.scalar.activation(o_a[:], psum[2 * NT + n][:], AF.Sigmoid,
                                     scale=SINV)
                os_t = epool.tile([128, PSH], f32, tag="ost", name=f"ost{n}")
                nc.vector.tensor_mul(os_t[:], o_a[:], th[n][:])
                nc.sync.dma_start(OS_d.ap()[n], os_t[:])

    nc.compile()
    return nc


def _get_nc():
    if "nc" not in _cache:
        _cache["nc"] = _build()
    return _cache["nc"]


def _prep_in_maps(x, hidden_s, hidden_u, hidden_c, Wd, Wu, Ww, Bd, Bu, Bw):
    # Activations, transposed: A_T[k, n], k = [hs (4096) | hu (512) | x (512)]
    A = np.concatenate(
        [hidden_s.reshape(B, S2), hidden_u, x], axis=1)                # [B, QC]
    A_kt = A.T.reshape(QT, 128, B)                                     # [q,p,n]
    A8 = np.ascontiguousarray(
        (A_kt * SA).astype(F8).transpose(1, 0, 2)).reshape(128, QT * B)
    AB = np.ascontiguousarray(
        A_kt.astype(BF).transpose(1, 0, 2)).reshape(128, QT * B)

    # Weights, transposed to [k, p]; contraction order [Wu | Ww | -Wd].
    WuT = Wu.reshape(4, S2, S2).transpose(0, 2, 1)                     # [4,S2,S2]
    WwT = Ww.reshape(4, S2, U).transpose(0, 2, 1)                      # [4,U,S2]
    WdT = (-Wd.reshape(4, S2, I)).transpose(0, 2, 1)                   # [4,I,S2]
    WT = np.concatenate([WuT, WwT, WdT], axis=1)                       # [4,QC,S2]
    W8_all = (WT[:3] * SW).astype(F8)                                  # [3,QC,S2]
    WB_all = WT[3].astype(BF)                                          # [QC,S2]

    bias = (Bw + Bd + Bu).reshape(4, S2).astype(np.float64)
    bias[:3] *= SA * SW
    hc = hidden_c.reshape(NT, 128, S2)

    in_maps = []
    for c in range(NCORES):
        p0 = c * PSH
        W8_c = np.ascontiguousarray(
            W8_all[:, :, p0:p0 + PSH].reshape(3, KP, 2, 128, PSH)
            .transpose(0, 3, 1, 2, 4)).reshape(3, 128, KP * 2 * PSH)
        WB_c = np.ascontiguousarray(
            WB_all[:, p0:p0 + PSH].reshape(QT, 128, PSH)
            .transpose(1, 0, 2)).reshape(128, QT * PSH)
        b_c = np.ascontiguousarray(
            bias[:, p0:p0 + PSH]).reshape(1, 4 * PSH).astype(BF)
        h_c = np.ascontiguousarray(
            hc[..., p0:p0 + PSH].transpose(1, 0, 2)).reshape(
                128, NT * PSH).astype(BF)
        in_maps.append({"A8": A8, "AB": AB, "W8": W8_c, "WB": WB_c,
                        "BIAS": b_c, "HC": h_c})
    return in_maps


def _run(inputs, trace=False, trace_kwargs=None):
    from concourse.bass_utils import run_bass_kernel_spmd

    nc = _get_nc()
    in_maps = _prep_in_maps(**inputs)
    res = run_bass_kernel_spmd(
        nc, in_maps, core_ids=list(range(NCORES)),
        trace=trace, **(trace_kwargs or {}))

    out_s = np.empty((B, S2), np.float32)
    c_out = np.empty((B, S2), np.float32)
    for c in range(NCORES):
        p0 = c * PSH
        out_s[:, p0:p0 + PSH] = res.results[c]["OS"].reshape(B, PSH)
        c_out[:, p0:p0 + PSH] = res.results[c]["CO"].reshape(B, PSH)
    return (out_s.reshape(B, 64, 64), c_out.reshape(B, 64, 64)), res


def kernel(**inputs):
    inputs = {k: np.asarray(v) for k, v in inputs.items()}
    (out_s, c_out), _ = _run(inputs)
    return (out_s, c_out)
